# Initial kernel scaffold
#
"""Multi-head attention distributed over 8 Trainium2 NeuronCores.

Sharding: core = (batch b, head-group g); each core computes 4 heads of
one batch end-to-end and returns a partial [2048, 512] output; the host
sums the two group partials per batch and adds the constant epilogue
vector bv @ Wo + bo (exact, since softmax rows sum to 1).

v4: all matmul operands are bf16 (host ships bf16 X^T and weights;
separate LDWEIGHTS amortizes/hides, input DMA halves). K^T is stored
per-head zero-padded ([128, 2048] with the other head's partition half
zeroed) so every matmul runs K=128 — no PE tiling-mode switches and no
partition shifts. PSUM accumulation and the softmax-sum/reciprocal path
stay f32/f32r for accuracy. Scores are computed transposed so the exp
output P^T feeds attn@V directly; a ones-column in V yields softmax
sums for free; 1/sum is applied via a PE outer-product broadcast.

Schedule: head-pair-0 K/Q projections sweep kt-outer over all 8 PSUM
banks behind the streaming input DMAs; V and head-pair-1 projections
run k-inner on the 2 out-proj/norm banks overlapping early attention.
Attention runs one (head, q-pair) unit at a time; normalization follows
each unit; output projection is split (heads 0/1 store, heads 2/3
CCE-accumulate into DRAM) and overlaps later attention.
"""

import numpy as np
import ml_dtypes

import concourse.bacc as bacc
import concourse.mybir as mybir
import concourse.tile as tile
from concourse.bass import ds
from concourse.bass_utils import run_bass_kernel_spmd

D_MODEL, DQ, DV, H = 512, 64, 64, 8
B, M = 4, 2048
NCORES, GROUPS = 8, 2
HL = H // GROUPS            # heads per core
VW = HL * (DV + 1)          # V width incl. ones columns = 260
SCALE = float(1.0 / np.sqrt(np.float32(M)))
NKT = D_MODEL // 128        # 4 contraction tiles over d_model
NTT = M // 128              # 16 token tiles
NQC = M // 512              # 4 query chunks of 512

F32 = mybir.dt.float32
F32R = mybir.dt.float32r
BF16 = mybir.dt.bfloat16
AF = mybir.ActivationFunctionType
OP = mybir.AluOpType

_prog_cache = {}


def _emit_body(nc, tc, t):
    P = 128

    with (
        tc.tile_pool(name="consts", bufs=1) as cpool,
        tc.tile_pool(name="persist", bufs=1) as ppool,
    ):
        wq_all = cpool.tile([P, NKT, 256], BF16, tag="wq", name="wq_all")
        wk_all = cpool.tile([P, NKT, 256], BF16, tag="wk", name="wk_all")
        wv_all = cpool.tile([P, NKT, VW], BF16, tag="wv", name="wv_all")
        wo_all = cpool.tile([P, HL, 512], BF16, tag="wo", name="wo_all")
        bmisc = cpool.tile([P, 264], F32, tag="bmisc", name="bmisc")
        misc = cpool.tile([1, 640], F32R, tag="misc", name="misc")
        bqk = bmisc[:, ds(0, 4)]
        onespat = bmisc[:, ds(4, VW)]
        ones = misc[ds(0, 1), ds(512, P)]

        for kt in range(NKT):
            nc.sync.dma_start(out=wk_all[:, kt, :], in_=t["wk"][ds(kt * P, P), :])
            nc.sync.dma_start(out=wq_all[:, kt, :], in_=t["wq"][ds(kt * P, P), :])
            nc.sync.dma_start(out=wv_all[:, kt, :], in_=t["wv"][ds(kt * P, P), :])
        for h in range(HL):
            nc.sync.dma_start(out=wo_all[ds(0, 64), h, :], in_=t["wo"][ds(h * 64, 64), :])
        nc.sync.dma_start(out=bmisc[:], in_=t["bmisc"][:, :])
        nc.sync.dma_start(out=misc[:], in_=t["misc"][:, :])
        nc.vector.memset(wo_all[ds(64, 64), :, :], 0.0)

        # persistent activations (kTh per-head zero-padded; o_sb zero-padded)
        qT = [ppool.tile([P, M], BF16, tag=f"qT{i}", name=f"qT{i}") for i in range(2)]
        kTh = [ppool.tile([P, M], BF16, tag=f"kTh{i}", name=f"kTh{i}") for i in range(HL)]
        v_all = ppool.tile([P, NTT, VW], BF16, tag="v", name="v_all")
        o_sb = [ppool.tile([P, M], BF16, tag=f"o{h}", name=f"osb{h}") for h in range(HL)]
        for h in range(HL):
            z0, z1 = (64, 64) if h % 2 == 0 else (0, 64)
            nc.vector.memset(kTh[h][ds(z0, z1), :], 0.0)
            nc.vector.memset(o_sb[h][ds(64, 64), :], 0.0)

        with tc.tile_pool(name="xc", bufs=8) as xc_pool:
            xch = {}
            # ---- head-pair-0 K and Q projections: kt-outer over 8 banks ----
            with tc.tile_pool(name="psq8", bufs=1, space="PSUM") as psq8:
                pss = {
                    (w, cg): psq8.tile([P, 1024], F32, tag=f"p{w}{cg}", name=f"p{w}{cg}")
                    for w in ("k", "q")
                    for cg in range(2)
                }
                for kt in range(NKT):
                    for w, w_all, xname in (("k", wk_all, "xkT"), ("q", wq_all, "xqT")):
                        c = xc_pool.tile([P, M], BF16, tag="xc", name="xc")
                        nc.sync.dma_start(out=c[:], in_=t[xname][ds(kt * P, P), :])
                        xch[(w, kt)] = c
                        for qc in range(NQC):
                            nc.tensor.matmul(
                                pss[(w, qc // 2)][:, ds((qc % 2) * 512, 512)],
                                lhsT=w_all[:, kt, ds(0, P)],
                                rhs=c[:, ds(qc * 512, 512)],
                                start=(kt == 0),
                                stop=(kt == NKT - 1),
                            )
                for cg in range(2):
                    nc.vector.tensor_scalar(
                        qT[0][:, ds(cg * 1024, 1024)],
                        pss[("q", cg)][:], bqk[:, ds(0, 1)], None, OP.add,
                    )
                    nc.vector.tensor_scalar(
                        kTh[0][ds(0, 64), ds(cg * 1024, 1024)],
                        pss[("k", cg)][ds(0, 64), :], bqk[ds(0, 64), ds(2, 1)], None, OP.add,
                    )
                    nc.vector.tensor_scalar(
                        kTh[1][ds(64, 64), ds(cg * 1024, 1024)],
                        pss[("k", cg)][ds(64, 64), :], bqk[ds(64, 64), ds(2, 1)], None, OP.add,
                    )

            with (
                tc.tile_pool(name="pT", bufs=6) as pt_pool,
                tc.tile_pool(name="fin", bufs=1) as fpool,
                tc.tile_pool(name="outb", bufs=3) as opool,
                tc.tile_pool(name="psatt", bufs=1, space="PSUM") as psa,
                tc.tile_pool(name="psfin", bufs=1, space="PSUM") as psf,
            ):
                sr = fpool.tile([P, P], F32R, tag="sr", name="sr")
                nc.scalar.activation(sr[ds(0, 1), ds(0, 1)], misc[ds(0, 1), ds(0, 1)], AF.Exp)

                # V projection: k-inner on the psf "f" bank
                xv = []
                for kt in range(NKT):
                    c = xc_pool.tile([P, M], BF16, tag="xv", name="xv", bufs=4)
                    nc.sync.dma_start(out=c[:], in_=t["xvT"][ds(kt * P, P), :])
                    xv.append(c)
                for tt in range(NTT):
                    psv = psf.tile([P, 512], F32, tag="f", name="pv")
                    for kt in range(NKT):
                        nc.tensor.matmul(
                            psv[:, ds(0, VW)],
                            lhsT=xv[kt][:, ds(tt * P, P)],
                            rhs=wv_all[:, kt, :],
                            start=(kt == 0),
                            stop=(kt == NKT - 1),
                        )
                    nc.vector.tensor_tensor(
                        v_all[:, tt, :], psv[:, ds(0, VW)], onespat[:], OP.add
                    )

                def proj_dq1():
                    # head-pair-1 K/Q projections: k-inner on the psf "rb" bank
                    for w, w_all, bcol in (("k", wk_all, 2), ("q", wq_all, 0)):
                        for qc in range(NQC):
                            ps = psf.tile([P, 512], F32, tag="rb", name="pp")
                            for kt in range(NKT):
                                nc.tensor.matmul(
                                    ps[:],
                                    lhsT=w_all[:, kt, ds(P, P)],
                                    rhs=xch[(w, kt)][:, ds(qc * 512, 512)],
                                    start=(kt == 0),
                                    stop=(kt == NKT - 1),
                                )
                            if w == "q":
                                nc.vector.tensor_scalar(
                                    qT[1][:, ds(qc * 512, 512)],
                                    ps[:], bqk[:, ds(1, 1)], None, OP.add,
                                )
                            else:
                                nc.vector.tensor_scalar(
                                    kTh[2][ds(0, 64), ds(qc * 512, 512)],
                                    ps[ds(0, 64), :], bqk[ds(0, 64), ds(3, 1)], None, OP.add,
                                )
                                nc.vector.tensor_scalar(
                                    kTh[3][ds(64, 64), ds(qc * 512, 512)],
                                    ps[ds(64, 64), :], bqk[ds(64, 64), ds(3, 1)], None, OP.add,
                                )

                def attn_unit(h, qcp):
                    hp = h // 2
                    po = [
                        psa.tile([65, 512], F32, tag=f"po{qci}", name=f"po{qci}")
                        for qci in range(2)
                    ]
                    for j in range(NTT):
                        sps = psa.tile([P, 1024], F32, tag="ps", name="ps", bufs=2)
                        for qci in range(2):
                            qc = qcp * 2 + qci
                            nc.tensor.matmul(
                                sps[:, ds(qci * 512, 512)],
                                lhsT=kTh[h][:, ds(j * P, P)],
                                rhs=qT[hp][:, ds(qc * 512, 512)],
                                start=True,
                                stop=True,
                            )
                        pt = pt_pool.tile([P, 1024], BF16, tag="pt", name="pt")
                        nc.scalar.activation(pt[:], sps[:], AF.Exp, scale=SCALE)
                        for qci in range(2):
                            nc.tensor.matmul(
                                po[qci][:],
                                lhsT=v_all[:, j, ds(h * 65, 65)],
                                rhs=pt[:, ds(qci * 512, 512)],
                                start=(j == 0),
                                stop=(j == NTT - 1),
                            )
                    for qci in range(2):
                        qc = qcp * 2 + qci
                        nc.vector.tensor_copy(
                            o_sb[h][ds(0, 64), ds(qc * 512, 512)], po[qci][ds(0, 64), :]
                        )
                        srow = fpool.tile([1, 512], F32R, tag="srow", name="srow", bufs=2)
                        nc.vector.tensor_copy(srow[:], po[qci][ds(64, 1), :])
                        idx = h * 4 + qc
                        nc.sync.dma_start(out=sr[:, ds(idx * 4, 4)], in_=srow[:])
                        with nc.allow_low_precision(reason="f32r == f32 bits"):
                            nc.vector.reciprocal(
                                sr[:, ds(64 + idx * 4, 4)], sr[:, ds(idx * 4, 4)]
                            )
                        rr = fpool.tile([1, 512], F32R, tag="rrow", name="rrow", bufs=2)
                        nc.sync.dma_start(out=rr[:], in_=sr[:, ds(64 + idx * 4, 4)])
                        rb = psf.tile([64, 512], F32, tag="rb", name="rb")
                        nc.tensor.matmul(
                            rb[:],
                            lhsT=ones[ds(0, 1), ds(0, 64)],
                            rhs=rr[ds(0, 1), :],
                            start=True,
                            stop=True,
                        )
                        nc.vector.tensor_tensor(
                            o_sb[h][ds(0, 64), ds(qc * 512, 512)],
                            o_sb[h][ds(0, 64), ds(qc * 512, 512)],
                            rb[:],
                            OP.mult,
                        )

                def outproj(tts, heads, accum):
                    for tt in tts:
                        fp = psf.tile([P, 512], F32, tag="f", name="f")
                        for i, h in enumerate(heads):
                            nc.tensor.matmul(
                                fp[:],
                                lhsT=o_sb[h][:, ds(tt * P, P)],
                                rhs=wo_all[:, h, :],
                                start=(i == 0),
                                stop=(i == len(heads) - 1),
                            )
                        ob = opool.tile([P, 512], F32, tag="ob", name="ob")
                        nc.vector.tensor_copy(ob[:], fp[:])
                        if accum:
                            nc.gpsimd.dma_start(
                                out=t["out"][ds(tt * P, P), :], in_=ob[:],
                                accum_op=OP.add,
                            )
                        else:
                            nc.sync.dma_start(out=t["out"][ds(tt * P, P), :], in_=ob[:])

                for h in range(HL):
                    for qcp in range(2):
                        attn_unit(h, qcp)
                        if h == 3 and qcp == 0:
                            outproj(range(8), (2, 3), True)
                    if h == 0:
                        proj_dq1()
                    if h == 1:
                        outproj(range(NTT), (0, 1), False)
                outproj(range(8, NTT), (2, 3), True)


def _build(reps=1):
    if reps in _prog_cache:
        return _prog_cache[reps]
    nc = bacc.Bacc(
        "TRN2",
        target_bir_lowering=False,
        debug=False,
        enable_asserts=False,
        num_devices=NCORES,
    )
    t = {}
    for name, shape, dt in (
        ("xqT", (D_MODEL, M), BF16),
        ("xkT", (D_MODEL, M), BF16),
        ("xvT", (D_MODEL, M), BF16),
        ("wq", (D_MODEL, 256), BF16),
        ("wk", (D_MODEL, 256), BF16),
        ("wv", (D_MODEL, VW), BF16),
        ("wo", (256, 512), BF16),
        ("bmisc", (128, 264), F32),
        ("misc", (1, 640), F32R),
    ):
        t[name] = nc.dram_tensor(name, shape, dt, kind="ExternalInput").ap()
    t["out"] = nc.dram_tensor("out", (M, D_MODEL), F32, kind="ExternalOutput").ap()

    with tile.TileContext(nc) as tc:
        for _ in range(reps):
            _emit_body(nc, tc, t)
    nc.compile()
    _prog_cache[reps] = (nc, t)
    return _prog_cache[reps]


def shard_inputs(query, key, value, Wq, bq, Wk, bk, Wv, bv, Wo, bo):
    query, key, value, Wq, bq, Wk, bk, Wv, bv, Wo, bo = (
        np.asarray(a, dtype=np.float32)
        for a in (query, key, value, Wq, bq, Wk, bk, Wv, bv, Wo, bo)
    )
    bf = ml_dtypes.bfloat16
    in_maps = []
    for b in range(B):
        xqT = np.ascontiguousarray(query[b].T).astype(bf)
        xkT = np.ascontiguousarray(key[b].T).astype(bf)
        xvT = np.ascontiguousarray(value[b].T).astype(bf)
        for g in range(GROUPS):
            hs = slice(g * 256, (g + 1) * 256)
            wv_ext = np.zeros((D_MODEL, VW), np.float32)
            onespat = np.zeros((VW,), np.float32)
            for i in range(HL):
                gh = g * HL + i
                wv_ext[:, i * 65 : i * 65 + 64] = Wv[:, gh * 64 : (gh + 1) * 64]
                onespat[i * 65 + 64] = 1.0
            bmisc = np.zeros((128, 264), np.float32)
            bmisc[:, 0:2] = bq[hs].reshape(2, 128).T
            bmisc[:, 2:4] = bk[hs].reshape(2, 128).T
            bmisc[:, 4:] = onespat
            misc = np.zeros((1, 640), np.float32)
            misc[0, 512:640] = 1.0
            in_maps.append(
                {
                    "xqT": xqT,
                    "xkT": xkT,
                    "xvT": xvT,
                    "wq": np.ascontiguousarray(Wq[:, hs]).astype(bf),
                    "wk": np.ascontiguousarray(Wk[:, hs]).astype(bf),
                    "wv": wv_ext.astype(bf),
                    "wo": np.ascontiguousarray(Wo[hs, :]).astype(bf),
                    "bmisc": bmisc,
                    "misc": misc,
                }
            )
    return in_maps


def unshard_outputs(results, c_epilogue):
    return np.stack(
        [
            results[2 * b]["out"] + results[2 * b + 1]["out"] + c_epilogue
            for b in range(B)
        ]
    )


def kernel(query, key, value, Wq, bq, Wk, bk, Wv, bv, Wo, bo):
    nc, _ = _build(reps=1)
    in_maps = shard_inputs(query, key, value, Wq, bq, Wk, bk, Wv, bv, Wo, bo)
    res = run_bass_kernel_spmd(nc, in_maps, core_ids=list(range(NCORES)))
    c = (
        np.asarray(bv, np.float32) @ np.asarray(Wo, np.float32)
        + np.asarray(bo, np.float32)
    ).astype(np.float32)
    return unshard_outputs(res.results, c)



# revision 1
# speedup vs baseline: 1.1250x; 1.1250x over previous
"""Multi-head attention distributed over 8 Trainium2 NeuronCores.

Sharding: core = (batch b, head-group g); each core computes 4 heads of
one batch end-to-end and returns a partial [2048, 512] output; the host
sums the two group partials per batch and adds the constant epilogue
vector bv @ Wo + bo (exact, since softmax rows sum to 1).

v4: all matmul operands are bf16 (host ships bf16 X^T and weights;
separate LDWEIGHTS amortizes/hides, input DMA halves). K^T is stored
per-head zero-padded ([128, 2048] with the other head's partition half
zeroed) so every matmul runs K=128 — no PE tiling-mode switches and no
partition shifts. PSUM accumulation and the softmax-sum/reciprocal path
stay f32/f32r for accuracy. Scores are computed transposed so the exp
output P^T feeds attn@V directly; a ones-column in V yields softmax
sums for free; 1/sum is applied via a PE outer-product broadcast.

Schedule: head-pair-0 K/Q projections sweep kt-outer over all 8 PSUM
banks behind the streaming input DMAs; V and head-pair-1 projections
run k-inner on the 2 out-proj/norm banks overlapping early attention.
Attention runs one (head, q-pair) unit at a time; normalization follows
each unit; output projection is split (heads 0/1 store, heads 2/3
CCE-accumulate into DRAM) and overlaps later attention.
"""

import numpy as np
import ml_dtypes

import concourse.bacc as bacc
import concourse.mybir as mybir
import concourse.tile as tile
from concourse.bass import ds
from concourse.bass_utils import run_bass_kernel_spmd

D_MODEL, DQ, DV, H = 512, 64, 64, 8
B, M = 4, 2048
NCORES, GROUPS = 8, 2
HL = H // GROUPS            # heads per core
VW = HL * (DV + 1)          # V width incl. ones columns = 260
SCALE = float(1.0 / np.sqrt(np.float32(M)))
NKT = D_MODEL // 128        # 4 contraction tiles over d_model
NTT = M // 128              # 16 token tiles
NQC = M // 512              # 4 query chunks of 512

F32 = mybir.dt.float32
F32R = mybir.dt.float32r
BF16 = mybir.dt.bfloat16
AF = mybir.ActivationFunctionType
OP = mybir.AluOpType

_prog_cache = {}


def _emit_body(nc, tc, t):
    P = 128

    with (
        tc.tile_pool(name="consts", bufs=1) as cpool,
        tc.tile_pool(name="persist", bufs=1) as ppool,
    ):
        wq_all = cpool.tile([P, NKT, 256], BF16, tag="wq", name="wq_all")
        wk_all = cpool.tile([P, NKT, 256], BF16, tag="wk", name="wk_all")
        wv_all = cpool.tile([P, NKT, VW], BF16, tag="wv", name="wv_all")
        wo_all = cpool.tile([P, HL, 512], BF16, tag="wo", name="wo_all")
        bmisc = cpool.tile([P, 264], F32, tag="bmisc", name="bmisc")
        misc = cpool.tile([1, 640], F32R, tag="misc", name="misc")
        bqk = bmisc[:, ds(0, 4)]
        onespat = bmisc[:, ds(4, VW)]
        ones = misc[ds(0, 1), ds(512, P)]

        for kt in range(NKT):
            nc.sync.dma_start(out=wk_all[:, kt, :], in_=t["wk"][ds(kt * P, P), :])
            nc.sync.dma_start(out=wq_all[:, kt, :], in_=t["wq"][ds(kt * P, P), :])
            nc.sync.dma_start(out=wv_all[:, kt, :], in_=t["wv"][ds(kt * P, P), :])
        for h in range(HL):
            nc.sync.dma_start(out=wo_all[ds(0, 64), h, :], in_=t["wo"][ds(h * 64, 64), :])
        nc.sync.dma_start(out=bmisc[:], in_=t["bmisc"][:, :])
        nc.sync.dma_start(out=misc[:], in_=t["misc"][:, :])
        nc.vector.memset(wo_all[ds(64, 64), :, :], 0.0)

        # persistent activations (kTh per-head zero-padded; o_sb zero-padded)
        qT = [ppool.tile([P, M], BF16, tag=f"qT{i}", name=f"qT{i}") for i in range(2)]
        kTh = [ppool.tile([P, M], BF16, tag=f"kTh{i}", name=f"kTh{i}") for i in range(HL)]
        v_all = ppool.tile([P, NTT, VW], BF16, tag="v", name="v_all")
        o_sb = [ppool.tile([P, M], BF16, tag=f"o{h}", name=f"osb{h}") for h in range(HL)]
        for h in range(HL):
            z0, z1 = (64, 64) if h % 2 == 0 else (0, 64)
            nc.vector.memset(kTh[h][ds(z0, z1), :], 0.0)
            nc.vector.memset(o_sb[h][ds(64, 64), :], 0.0)

        with tc.tile_pool(name="xc", bufs=8) as xc_pool:
            xch = {}
            # ---- head-pair-0 K and Q projections: kt-outer over 8 banks ----
            with tc.tile_pool(name="psq8", bufs=1, space="PSUM") as psq8:
                pss = {
                    (w, cg): psq8.tile([P, 1024], F32, tag=f"p{w}{cg}", name=f"p{w}{cg}")
                    for w in ("k", "q")
                    for cg in range(2)
                }
                for kt in range(NKT):
                    for w, w_all, xname in (("k", wk_all, "xkT"), ("q", wq_all, "xqT")):
                        c = xc_pool.tile([P, M], BF16, tag="xc", name="xc")
                        nc.sync.dma_start(out=c[:], in_=t[xname][ds(kt * P, P), :])
                        xch[(w, kt)] = c
                        for qc in range(NQC):
                            nc.tensor.matmul(
                                pss[(w, qc // 2)][:, ds((qc % 2) * 512, 512)],
                                lhsT=w_all[:, kt, ds(0, P)],
                                rhs=c[:, ds(qc * 512, 512)],
                                start=(kt == 0),
                                stop=(kt == NKT - 1),
                            )
                for cg in range(2):
                    nc.vector.tensor_scalar(
                        qT[0][:, ds(cg * 1024, 1024)],
                        pss[("q", cg)][:], bqk[:, ds(0, 1)], None, OP.add,
                    )
                    nc.vector.tensor_scalar(
                        kTh[0][ds(0, 64), ds(cg * 1024, 1024)],
                        pss[("k", cg)][ds(0, 64), :], bqk[ds(0, 64), ds(2, 1)], None, OP.add,
                    )
                    nc.vector.tensor_scalar(
                        kTh[1][ds(64, 64), ds(cg * 1024, 1024)],
                        pss[("k", cg)][ds(64, 64), :], bqk[ds(64, 64), ds(2, 1)], None, OP.add,
                    )

            with (
                tc.tile_pool(name="pT", bufs=6) as pt_pool,
                tc.tile_pool(name="fin", bufs=1) as fpool,
                tc.tile_pool(name="outb", bufs=3) as opool,
                tc.tile_pool(name="psatt", bufs=1, space="PSUM") as psa,
                tc.tile_pool(name="psfin", bufs=1, space="PSUM") as psf,
            ):
                sr = fpool.tile([P, P], F32R, tag="sr", name="sr")
                nc.scalar.activation(sr[ds(0, 1), ds(0, 1)], misc[ds(0, 1), ds(0, 1)], AF.Exp)

                # V projection: k-inner on the psf "f" bank
                xv = []
                for kt in range(NKT):
                    c = xc_pool.tile([P, M], BF16, tag="xv", name="xv", bufs=4)
                    nc.sync.dma_start(out=c[:], in_=t["xvT"][ds(kt * P, P), :])
                    xv.append(c)
                for tt in range(NTT):
                    psv = psf.tile([P, 512], F32, tag="f", name="pv")
                    for kt in range(NKT):
                        nc.tensor.matmul(
                            psv[:, ds(0, VW)],
                            lhsT=xv[kt][:, ds(tt * P, P)],
                            rhs=wv_all[:, kt, :],
                            start=(kt == 0),
                            stop=(kt == NKT - 1),
                        )
                    nc.vector.tensor_tensor(
                        v_all[:, tt, :], psv[:, ds(0, VW)], onespat[:], OP.add
                    )

                def proj_dq1():
                    # head-pair-1 K/Q projections: k-inner on the psf "rb" bank
                    for w, w_all, bcol in (("k", wk_all, 2), ("q", wq_all, 0)):
                        for qc in range(NQC):
                            ps = psf.tile([P, 512], F32, tag="rb", name="pp")
                            for kt in range(NKT):
                                nc.tensor.matmul(
                                    ps[:],
                                    lhsT=w_all[:, kt, ds(P, P)],
                                    rhs=xch[(w, kt)][:, ds(qc * 512, 512)],
                                    start=(kt == 0),
                                    stop=(kt == NKT - 1),
                                )
                            if w == "q":
                                nc.vector.tensor_scalar(
                                    qT[1][:, ds(qc * 512, 512)],
                                    ps[:], bqk[:, ds(1, 1)], None, OP.add,
                                )
                            else:
                                nc.vector.tensor_scalar(
                                    kTh[2][ds(0, 64), ds(qc * 512, 512)],
                                    ps[ds(0, 64), :], bqk[ds(0, 64), ds(3, 1)], None, OP.add,
                                )
                                nc.vector.tensor_scalar(
                                    kTh[3][ds(64, 64), ds(qc * 512, 512)],
                                    ps[ds(64, 64), :], bqk[ds(64, 64), ds(3, 1)], None, OP.add,
                                )

                def attn_unit(h, qcp):
                    hp = h // 2
                    po = [
                        psa.tile([65, 512], F32, tag=f"po{qci}", name=f"po{qci}")
                        for qci in range(2)
                    ]
                    for j in range(NTT):
                        sps = psa.tile([P, 1024], F32, tag="ps", name="ps", bufs=2)
                        for qci in range(2):
                            qc = qcp * 2 + qci
                            nc.tensor.matmul(
                                sps[:, ds(qci * 512, 512)],
                                lhsT=kTh[h][:, ds(j * P, P)],
                                rhs=qT[hp][:, ds(qc * 512, 512)],
                                start=True,
                                stop=True,
                            )
                        pt = pt_pool.tile([P, 1024], BF16, tag="pt", name="pt")
                        nc.scalar.activation(pt[:], sps[:], AF.Exp, scale=SCALE)
                        for qci in range(2):
                            nc.tensor.matmul(
                                po[qci][:],
                                lhsT=v_all[:, j, ds(h * 65, 65)],
                                rhs=pt[:, ds(qci * 512, 512)],
                                start=(j == 0),
                                stop=(j == NTT - 1),
                            )
                    for qci in range(2):
                        qc = qcp * 2 + qci
                        nc.vector.tensor_copy(
                            o_sb[h][ds(0, 64), ds(qc * 512, 512)], po[qci][ds(0, 64), :]
                        )
                        srow = fpool.tile([1, 512], F32R, tag="srow", name="srow", bufs=2)
                        nc.vector.tensor_copy(srow[:], po[qci][ds(64, 1), :])
                        idx = h * 4 + qc
                        nc.sync.dma_start(out=sr[:, ds(idx * 4, 4)], in_=srow[:])
                        with nc.allow_low_precision(reason="f32r == f32 bits"):
                            nc.vector.reciprocal(
                                sr[:, ds(64 + idx * 4, 4)], sr[:, ds(idx * 4, 4)]
                            )
                        rr = fpool.tile([1, 512], F32R, tag="rrow", name="rrow", bufs=2)
                        nc.sync.dma_start(out=rr[:], in_=sr[:, ds(64 + idx * 4, 4)])
                        rb = psf.tile([64, 512], F32, tag="rb", name="rb")
                        nc.tensor.matmul(
                            rb[:],
                            lhsT=ones[ds(0, 1), ds(0, 64)],
                            rhs=rr[ds(0, 1), :],
                            start=True,
                            stop=True,
                        )
                        nc.vector.tensor_tensor(
                            o_sb[h][ds(0, 64), ds(qc * 512, 512)],
                            o_sb[h][ds(0, 64), ds(qc * 512, 512)],
                            rb[:],
                            OP.mult,
                        )

                def outproj(tts, heads, accum):
                    for tt in tts:
                        fp = psf.tile([P, 512], F32, tag="f", name="f")
                        for i, h in enumerate(heads):
                            nc.tensor.matmul(
                                fp[:],
                                lhsT=o_sb[h][:, ds(tt * P, P)],
                                rhs=wo_all[:, h, :],
                                start=(i == 0),
                                stop=(i == len(heads) - 1),
                            )
                        ob = opool.tile([P, 512], F32, tag="ob", name="ob")
                        nc.vector.tensor_copy(ob[:], fp[:])
                        if accum:
                            nc.gpsimd.dma_start(
                                out=t["out"][ds(tt * P, P), :], in_=ob[:],
                                accum_op=OP.add,
                            )
                        else:
                            nc.sync.dma_start(out=t["out"][ds(tt * P, P), :], in_=ob[:])

                for h in range(HL):
                    for qcp in range(2):
                        attn_unit(h, qcp)
                        if h == 3 and qcp == 0:
                            outproj(range(8), (2, 3), True)
                    if h == 0:
                        proj_dq1()
                    if h == 1:
                        outproj(range(NTT), (0, 1), False)
                outproj(range(8, NTT), (2, 3), True)


def _build(reps=1):
    if reps in _prog_cache:
        return _prog_cache[reps]
    nc = bacc.Bacc(
        "TRN2",
        target_bir_lowering=False,
        debug=False,
        enable_asserts=False,
        num_devices=NCORES,
    )
    t = {}
    for name, shape, dt in (
        ("xqT", (D_MODEL, M), BF16),
        ("xkT", (D_MODEL, M), BF16),
        ("xvT", (D_MODEL, M), BF16),
        ("wq", (D_MODEL, 256), BF16),
        ("wk", (D_MODEL, 256), BF16),
        ("wv", (D_MODEL, VW), BF16),
        ("wo", (256, 512), BF16),
        ("bmisc", (128, 264), F32),
        ("misc", (1, 640), F32R),
    ):
        t[name] = nc.dram_tensor(name, shape, dt, kind="ExternalInput").ap()
    t["out"] = nc.dram_tensor("out", (M, D_MODEL), F32, kind="ExternalOutput").ap()

    with tile.TileContext(nc) as tc:
        for _ in range(reps):
            _emit_body(nc, tc, t)
    nc.compile()
    _prog_cache[reps] = (nc, t)
    return _prog_cache[reps]


def shard_inputs(query, key, value, Wq, bq, Wk, bk, Wv, bv, Wo, bo):
    query, key, value, Wq, bq, Wk, bk, Wv, bv, Wo, bo = (
        np.asarray(a, dtype=np.float32)
        for a in (query, key, value, Wq, bq, Wk, bk, Wv, bv, Wo, bo)
    )
    bf = ml_dtypes.bfloat16
    in_maps = []
    for b in range(B):
        xqT = np.ascontiguousarray(query[b].T).astype(bf)
        xkT = np.ascontiguousarray(key[b].T).astype(bf)
        xvT = np.ascontiguousarray(value[b].T).astype(bf)
        for g in range(GROUPS):
            hs = slice(g * 256, (g + 1) * 256)
            wv_ext = np.zeros((D_MODEL, VW), np.float32)
            onespat = np.zeros((VW,), np.float32)
            for i in range(HL):
                gh = g * HL + i
                wv_ext[:, i * 65 : i * 65 + 64] = Wv[:, gh * 64 : (gh + 1) * 64]
                onespat[i * 65 + 64] = 1.0
            bmisc = np.zeros((128, 264), np.float32)
            bmisc[:, 0:2] = bq[hs].reshape(2, 128).T
            bmisc[:, 2:4] = bk[hs].reshape(2, 128).T
            bmisc[:, 4:] = onespat
            misc = np.zeros((1, 640), np.float32)
            misc[0, 512:640] = 1.0
            in_maps.append(
                {
                    "xqT": xqT,
                    "xkT": xkT,
                    "xvT": xvT,
                    "wq": np.ascontiguousarray(Wq[:, hs]).astype(bf),
                    "wk": np.ascontiguousarray(Wk[:, hs]).astype(bf),
                    "wv": wv_ext.astype(bf),
                    "wo": np.ascontiguousarray(Wo[hs, :]).astype(bf),
                    "bmisc": bmisc,
                    "misc": misc,
                }
            )
    return in_maps


def unshard_outputs(results, c_epilogue):
    return np.stack(
        [
            results[2 * b]["out"] + results[2 * b + 1]["out"] + c_epilogue
            for b in range(B)
        ]
    )


def kernel(query, key, value, Wq, bq, Wk, bk, Wv, bv, Wo, bo):
    nc, _ = _build(reps=1)
    in_maps = shard_inputs(query, key, value, Wq, bq, Wk, bk, Wv, bv, Wo, bo)
    res = run_bass_kernel_spmd(nc, in_maps, core_ids=list(range(NCORES)))
    c = (
        np.asarray(bv, np.float32) @ np.asarray(Wo, np.float32)
        + np.asarray(bo, np.float32)
    ).astype(np.float32)
    return unshard_outputs(res.results, c)

